# revision 3
# baseline (speedup 1.0000x reference)
"""Chamfer distance kernel for 8 TRN2 NeuronCores (v3).

Problem: x, y of shape (8, 8192, 3) f32; output scalar
  sum_b max(mean_n min_m ||x_bn - y_bm||, mean_m min_n ||x_bn - y_bm||)

Sharding: batch-parallel, one batch element per core (B == n_cores == 8).
Each core computes its batch's scalar max(mean1, mean2); the host sums the
8 per-core scalars (the hint's single all-reduce, done at gather time).

Per-core algorithm (single matmul sweep, both directions):
  The TensorEngine computes P[n, m] = x~.y~ - xx/2 - yy/2 = -dist^2/2 via a
  K=16 fp16 matmul per [128, 2048] PSUM group: each f32 coordinate is split
  into an fp16 hi/lo pair (exact to ~2^-22 rel), all four cross products are
  K-rows, and the point norms ride along against constant-one rows.  fp16
  products accumulate exactly in fp32 PSUM.  min_m dist^2 = -2 max_m P and
  sqrt is monotonic, so both chamfer directions are max-reductions over P.
  Per group, ACT casts PSUM to fp16 (E), DVE row-maxes E (direction 1) and
  max-accumulates E into a [128, n] ACC tile (direction 2, finished by one
  transposed partition-axis reduction at the end).

  v3 changes vs v2 (whose non-sweep time dominated the measured wall):
  1. DMA consolidation.  Each small DMA has ~2 us fixed completion latency
     on HW.  v2 issued ~75 DMAs outside the sweep (64 single-tile
     transposes, 8 SEL partition-expands, strided input loads emitting
     8192 12-byte descriptors).  v3: one [128, 4096] transpose DMA per
     input (the xbar transposes 128-col tiles independently, so one big
     DMA produces the identical layout), 2 SEL DMAs total (SELX/SELY share
     one [128, 256] tile; only quadrant bases 0/64 are populated since
     quads=2 never reads 32/96), and contiguous input loads
     ("(p t) d -> p (t d)": 768B/partition descriptors; the implied point
     relabeling is invisible to min/mean reductions).
  2. Dead work removal.  Dropped the unused negt/junkg tiles, the junk16
     init (write-only target), the d-major staging copy (hi slots are
     written straight from the t-major load via a strided view), and the
     full [128, 4096] V memset (only K-slots 9..15, read as stationary
     zeros, need clearing).  Square moved from ACT to DVE so ACT loads a
     single activation table set (sqrt) instead of two.
  3. ACC init by first cast.  At t=0 ACT casts PSUM directly into the ACC
     tile (no -inf memset of [128, 8192], no t=0 max pass).
"""

import numpy as np
from contextlib import ExitStack

B = 8
NPOINTS = 8192
EPS = 1e-10
GROUP_FD = 2048
CHUNK = 512
PSUM_BUFS = 2
H16_BUFS = 6


def emit(tc, out_ap, x_ap, y_ap, n=NPOINTS, group_fd=GROUP_FD,
         psum_bufs=PSUM_BUFS, h16_bufs=H16_BUFS, chunk=CHUNK, reps=1,
         ablate=None, tag=""):
    """Emit the per-core chamfer kernel into TileContext tc.

    x_ap, y_ap: DRAM [n, 3] f32.  out_ap: DRAM [1, 1] f32.
    """
    import concourse.mybir as mybir
    from concourse.mybir import AluOpType as alu

    nc = tc.nc
    f32 = mybir.dt.float32
    f16 = mybir.dt.float16
    X = mybir.AxisListType.X
    ntile = n // 128
    groups = n // group_fd
    nq = group_fd // chunk

    ctx = ExitStack()
    with ctx:
        singles = ctx.enter_context(tc.tile_pool(name="singles" + tag, bufs=1))
        work = ctx.enter_context(tc.tile_pool(name="work" + tag, bufs=2))
        h16p = ctx.enter_context(tc.tile_pool(name="h16p" + tag, bufs=h16_bufs))
        colp = ctx.enter_context(tc.tile_pool(name="colp" + tag, bufs=2))
        psum = ctx.enter_context(tc.tile_pool(name="psum" + tag, bufs=psum_bufs,
                                              space="PSUM"))

        XW = singles.tile([128, n], f16, tag="XW")
        YW = singles.tile([128, n], f16, tag="YW")
        D1 = singles.tile([128, ntile], f32, tag="D1")
        D2 = singles.tile([128, ntile], f32, tag="D2")
        ACCall = singles.tile([128, n], f16, tag="ACCall")
        junk16 = singles.tile([128, group_fd], f16, tag="junk16")
        pmaxs = singles.tile([128, ntile], f16, tag="pmaxs")
        eps_col = singles.tile([128, 1], f32, tag="eps_col")
        nc.vector.memset(eps_col, EPS)

        # Selection stationaries for the prep gather matmuls.  The per-point
        # vectors live (after transpose) at partitions (t%2)*64 + v, v being
        # the vector-slot index:  [h0 h1 h2 l0 l1 l2 nh nl one, 0...].  The
        # K=16 W-row pattern at psum row i is vec_{vmap[i % 32]} (>=16 -> 0):
        #   XW rows: [xh0..xh2 xl0..xl2 | xh0..xh2 xl0..xl2 | 1 1 | nxh nxl]
        #   YW rows: [yh0..yh2 yl0..yl2 | yl0..yl2 yh0..yh2 | nyh nyl | 1 1]
        # pairing k: 0-2 (xh,yh)d, 3-5 (xl,yl)d, 6-8 (xh,yl)d, 9-11 (xl,yh)d,
        # 12 (1,nyh), 13 (1,nyl), 14 (nxh,1), 15 (nxl,1).
        VMAP_X = [0, 1, 2, 3, 4, 5, 0, 1, 2, 3, 4, 5, 8, 8, 6, 7]
        VMAP_Y = [0, 1, 2, 3, 4, 5, 3, 4, 5, 0, 1, 2, 6, 7, 8, 8]

        # SELX/SELY share one [128, 256] tile filled from one [1, 32*256]
        # content row: SELXY row v = [ SELX cols (4 q-copies of 32) | SELY ].
        # Engines can't write partition-strided single rows, so content is
        # laid out in partition 0 and partition-expanded by one DMA to rows
        # 0..31, then one SBUF->SBUF DMA replicates to base 64 (the only
        # other quadrant base the K=16 stationaries read).
        SELXY = singles.tile([128, 256], f16, tag="SELXY")
        content = singles.tile([1, 32 * 256], f16, tag="selc")
        nc.vector.memset(content, 0.0)
        cw = content[:, :].rearrange("p (v w r) -> p v w r", w=8, r=32)
        for off, vmap in ((0, VMAP_X), (4, VMAP_Y)):
            by_v = {}
            for r, v in enumerate(vmap):
                by_v.setdefault(v, []).append(r)
            for v, rs in by_v.items():
                start = prev = rs[0]
                for r in rs[1:] + [None]:
                    if r is not None and r == prev + 1:
                        prev = r
                        continue
                    nc.vector.memset(cw[:, v, off:off + 4, start:prev + 1], 1.0)
                    if r is not None:
                        start = prev = r
        nc.sync.dma_start(out=SELXY[0:32, :], in_=content[0:1, :])
        nc.sync.dma_start(out=SELXY[64:96, :], in_=SELXY[0:32, :])
        SELX = SELXY[:, 0:128]
        SELY = SELXY[:, 128:256]

        def prep(inp, W, sel):
            """Build W [128, n] fp16 (16 K-rows replicated at partition
            bases 0/64) without partition-collapsing DMAs: compute the
            9 per-point vectors, transpose them with one xbar DMA, then
            gather+replicate into W via selection matmuls + PSUM cast."""
            # contiguous load: partition p holds points p*ntile + t,
            # cols (t d).  Point identity is a pure relabeling that the
            # min/mean reductions never observe.
            Xw = work.tile([128, 3 * ntile], f32, tag="Xw")
            nc.sync.dma_start(
                out=Xw,
                in_=inp.rearrange("(p t) d -> p (t d)", p=128),
            )
            Xw3 = Xw[:, :].rearrange("p (t d) -> p d t", d=3)
            # V: 64 vector slots per tile, col order t*64 + v.  64 slots (not
            # 32) so the transposed slot rows land at partition bases {0, 64}
            # only: 32/96-base PE tiles fail at runtime on this silicon.
            V = work.tile([128, 64 * ntile], f16, tag="V")
            nc.vector.memset(V, 0.0)
            v32 = V[:, :].rearrange("p (t v) -> p v t", v=64)
            nc.vector.tensor_copy(v32[:, 0:3, :], Xw3)               # xh
            nc.vector.tensor_tensor(v32[:, 3:6, :], Xw3, v32[:, 0:3, :],
                                    alu.subtract)                    # xl
            Sq = work.tile([128, 3 * ntile], f32, tag="Sq")
            nc.vector.tensor_tensor(Sq, Xw, Xw, alu.mult)
            sq3 = Sq[:, :].rearrange("p (t d) -> p d t", d=3)
            nxx = work.tile([128, ntile], f32, tag="nxx")
            nc.vector.tensor_tensor(nxx, sq3[:, 0, :], sq3[:, 1, :], alu.add)
            nc.vector.tensor_tensor(nxx, nxx, sq3[:, 2, :], alu.add)
            nc.vector.tensor_scalar_mul(nxx, nxx, -0.5)
            nc.scalar.copy(v32[:, 6, :], nxx)                        # nh
            nc.vector.tensor_tensor(v32[:, 7, :], nxx, v32[:, 6, :],
                                    alu.subtract)                    # nl
            nc.vector.memset(v32[:, 8, :], 1.0)                      # ones
            if ablate == "prepV":
                return V
            # transpose: TV[(t%2)*64 + v, (t//2)*128 + p] = vec_v[point(p,t)]
            # (one xbar DMA; it transposes 128-col tiles independently).
            TV = work.tile([128, (ntile // 2) * 128], f16, tag="TV")
            nc.sync.dma_start_transpose(
                TV[:, :].rearrange("p (c j) -> p c j", j=128), V)
            if ablate == "prepTV":
                return TV
            # gather+replicate via selection matmuls, cast PSUM -> W.  Each
            # matmul streams a [16, 512] moving block into one full PSUM
            # bank: TV's base-0 rows hold the even tiles of an 8-tile group
            # contiguously, base-64 rows the odd tiles.  W's 128-col blocks
            # come out in (even..., odd...) permuted tile order — again a
            # pure point relabeling.
            for c in range(n // group_fd):
                ps = psum.tile([128, group_fd], f32, tag="ps")
                for j in range(group_fd // 512):
                    b = c * (group_fd // 512) + j
                    q = 64 * (b % 2)
                    nc.tensor.matmul(
                        ps[:, j * 512:(j + 1) * 512],
                        sel[q:q + 16, :],
                        TV[q:q + 16, (b // 2) * 512:(b // 2) * 512 + 512],
                        start=True, stop=True, tile_position=(q, 0),
                    )
                if c % 2 == 0:
                    nc.scalar.copy(W[:, c * group_fd:(c + 1) * group_fd], ps)
                else:
                    nc.vector.tensor_copy(W[:, c * group_fd:(c + 1) * group_fd],
                                          ps)

        def dbg_out(src_f16):
            dbg = singles.tile([1, 1], f32, tag="dbg")
            nc.vector.tensor_copy(dbg, src_f16)
            nc.sync.dma_start(out=out_ap, in_=dbg)

        if ablate == "sel":
            dbg_out(SELXY[0:1, 0:1])
            return
        if ablate in ("prepV", "prepTV"):
            probe = prep(x_ap, XW, SELX)
            dbg_out(probe[0:1, 0:1])
            return
        prep(x_ap, XW, SELX)
        if ablate == "prep1":
            dbg_out(XW[0:1, 0:1])
            return
        prep(y_ap, YW, SELY)
        if ablate == "prep2":
            dbg_out(YW[0:1, 0:1])
            return

        # ---- main sweep: one matmul pass ----
        # Per 2048-col group: PE fills a PSUM group (2-quadrant K=16
        # matmuls, 512-col bank-aligned chunks); ACT alone drains PSUM,
        # casting to fp16 — into ACCall directly at t=0 (initializing the
        # direction-2 accumulator), into an E tile otherwise.  DVE row-maxes
        # the cast (tensor_scalar junk write at 4x rate, accum_out into
        # gcols; accum_out must target a small pool tile) and, for t>0,
        # max-accumulates E into ACCall.
        gi = 0
        for _rep in range(reps):
          for t in range(ntile):
              gcols = colp.tile([128, groups], f16, tag="gcols")
              for g in range(groups):
                  ps = psum.tile([128, group_fd], f32, tag="ps")
                  for c in range(nq):
                      m0 = g * group_fd + c * chunk
                      qi = gi * nq + c
                      q = 64 * (qi % 2)
                      nc.tensor.matmul(
                          ps[:, c * chunk:(c + 1) * chunk],
                          XW[q:q + 16, t * 128:(t + 1) * 128],
                          YW[q:q + 16, m0:m0 + chunk],
                          start=True, stop=True,
                          tile_position=(q, 0),
                      )
                  gi += 1
                  if t == 0:
                      dst = ACCall[:, g * group_fd:(g + 1) * group_fd]
                      nc.scalar.copy(dst, ps[:, :])
                      nc.vector.tensor_scalar(junk16, dst, 0.0, None,
                                              alu.min, alu.max,
                                              accum_out=gcols[:, g:g + 1])
                  else:
                      E = h16p.tile([128, group_fd], f16, tag="E")
                      nc.scalar.copy(E, ps[:, :])
                      nc.vector.tensor_scalar(junk16, E, 0.0, None,
                                              alu.min, alu.max,
                                              accum_out=gcols[:, g:g + 1])
                      nc.vector.tensor_tensor(
                          ACCall[:, g * group_fd:(g + 1) * group_fd],
                          ACCall[:, g * group_fd:(g + 1) * group_fd],
                          E, alu.max)
              # direction-1 per-t finish (regular write, no ACT involvement)
              nc.vector.tensor_reduce(pmaxs[:, t:t + 1], gcols[:, 0:groups],
                                      axis=X, op=alu.max)

        # D1 = sqrt(-2*max + EPS), one activation for all tiles.  Clamp
        # the maxima to <= 0 first (guards sqrt against representation
        # noise on near-duplicate points).
        pm2 = colp.tile([128, ntile], f16, tag="pm2")
        nc.vector.tensor_scalar(pm2, pmaxs, 0.0, None, alu.min, alu.bypass)
        nc.scalar.activation(D1[:, :], pm2[:, :],
                             mybir.ActivationFunctionType.Sqrt,
                             bias=eps_col[:, :], scale=-2.0)

        # ---- direction-2 tail: transpose ACC once, clamp, reduce, sqrt ----
        GB = colp.tile([128, ntile], f16, tag="GB")
        tch = 2048
        for g in range(n // tch):
            tp = h16p.tile([128, tch], f16, tag="tp")
            nc.sync.dma_start_transpose(
                tp[:, :].rearrange("p (c j) -> p c j", j=128),
                ACCall[:, g * tch:(g + 1) * tch])
            jg = h16p.tile([128, tch], f16, tag="jg")
            nc.vector.tensor_scalar(jg, tp, 0.0, None, alu.min, alu.bypass)
            nc.vector.tensor_reduce(
                GB[:, g * (tch // 128):(g + 1) * (tch // 128)],
                jg[:, :].rearrange("p (c j) -> p c j", j=128),
                axis=X, op=alu.max)
        nc.scalar.activation(D2[:, :], GB[:, :],
                             mybir.ActivationFunctionType.Sqrt,
                             bias=eps_col[:, :], scale=-2.0)

        # ---- mean over points, max of the two directions, write out ----
        sums = singles.tile([128, 2], f32, tag="sums")
        nc.vector.tensor_reduce(sums[:, 0:1], D1[:, :], axis=X, op=alu.add)
        nc.vector.tensor_reduce(sums[:, 1:2], D2[:, :], axis=X, op=alu.add)
        ones = singles.tile([128, 1], f32, tag="ones")
        nc.vector.memset(ones, 1.0)
        pstail = psum.tile([128, group_fd], f32, tag="ps")
        pq = pstail[0:1, 0:2]
        nc.tensor.matmul(pq, ones[:, :], sums[:, :], start=True, stop=True)
        fin = singles.tile([1, 2], f32, tag="fin")
        res = singles.tile([1, 1], f32, tag="res")
        nc.vector.tensor_scalar(fin, pq, 1.0 / n, None, alu.mult, alu.max,
                                accum_out=res)
        nc.sync.dma_start(out=out_ap, in_=res)


_NC_CACHE = {}


def build(n=NPOINTS, reps=1, body_reps=1, group_fd=GROUP_FD,
          psum_bufs=PSUM_BUFS, h16_bufs=H16_BUFS, chunk=CHUNK, ablate=None):
    key = (n, reps, body_reps, group_fd, psum_bufs, h16_bufs, chunk, ablate)
    if key in _NC_CACHE:
        return _NC_CACHE[key]
    import concourse.mybir as mybir
    import concourse.tile as tile
    from concourse import bacc

    nc = bacc.Bacc(None, target_bir_lowering=False)
    x = nc.dram_tensor("x", [n, 3], mybir.dt.float32, kind="ExternalInput")
    y = nc.dram_tensor("y", [n, 3], mybir.dt.float32, kind="ExternalInput")
    out = nc.dram_tensor("out", [1, 1], mybir.dt.float32, kind="ExternalOutput")
    with tile.TileContext(nc) as tc:
        for r in range(body_reps):
            emit(tc, out[:, :], x[:, :], y[:, :], n=n, group_fd=group_fd,
                 psum_bufs=psum_bufs, h16_bufs=h16_bufs, chunk=chunk,
                 reps=reps, ablate=ablate, tag=f"r{r}" if r else "")
    nc.finalize()
    _NC_CACHE[key] = nc
    return nc


def kernel(x, y):
    """Full-input entry point: x, y (8, 8192, 3) f32 -> scalar f32."""
    from concourse.bass_utils import run_bass_kernel_spmd

    x = np.asarray(x, dtype=np.float32)
    y = np.asarray(y, dtype=np.float32)
    assert x.shape == (B, NPOINTS, 3) and y.shape == (B, NPOINTS, 3)
    nc = build()
    in_maps = [
        {"x": np.ascontiguousarray(x[b]), "y": np.ascontiguousarray(y[b])}
        for b in range(B)
    ]
    res = run_bass_kernel_spmd(nc, in_maps, core_ids=list(range(B)))
    total = np.float32(0.0)
    for r in res.results:
        total = np.float32(total + np.float32(r["out"][0, 0]))
    return total


# revision 20
# speedup vs baseline: 1.1630x; 1.1630x over previous
"""Chamfer distance kernel for 8 TRN2 NeuronCores (v3).

Problem: x, y of shape (8, 8192, 3) f32; output scalar
  sum_b max(mean_n min_m ||x_bn - y_bm||, mean_m min_n ||x_bn - y_bm||)

Sharding: batch-parallel, one batch element per core (B == n_cores == 8).
Each core computes its batch's scalar max(mean1, mean2); the host sums the
8 per-core scalars (the hint's single all-reduce, done at gather time).

Per-core algorithm (single matmul sweep, both directions):
  The TensorEngine computes P[n, m] = x~.y~ - xx/2 - yy/2 = -dist^2/2 via a
  K=16 fp16 matmul per [128, 2048] PSUM group: each f32 coordinate is split
  into an fp16 hi/lo pair (exact to ~2^-22 rel), all four cross products are
  K-rows, and the point norms ride along against constant-one rows.  fp16
  products accumulate exactly in fp32 PSUM.  ACT drains each PSUM group
  with relu(-P) = d^2/2 (clamp >= 0 for free) into a contiguous [128, n]
  fp16 strip per t-tile; DVE runs one whole-strip row-min (direction 1,
  tensor_scalar junk write at 4x with accum_out) and one whole-strip
  min-accumulate into a double-buffered [128, n] ACC (direction 2,
  finished by a transposed partition-axis reduction at the end).  sqrt is
  monotonic so all reductions happen on d^2/2.

  Keys facts this structure is built on (HW-measured, see memory notes):
  - Each small DMA costs ~2 us fixed on HW: prep does ONE contiguous load
    + ONE [128, 4096] xbar transpose per input and 2 SEL DMAs total (v2
    did ~75 small DMAs).  Input point order is relabeled by the loads;
    min/mean reductions never observe it.
  - The sweep is jointly ACT/DVE-bound (PE ~25%): ACT [128,2048] PSUM
    casts run ~2.5 us back-to-back; DVE fp16 [128, n] tensor_scalar ~4x,
    tensor_tensor ~2x, but IN-PLACE DVE ops (out aliasing an input) drop
    to 1x on HW — hence the junk-output row-min and the ping-pong ACC.
  - Whole-strip DVE ops (1 per t instead of 4 per t) cut sync edges;
    direction-1 only needs the min over ALL m per x-point.
  - accum_out must target small column offsets: row minima batch 8-wide
    in a colp tile, then one copy to pmaxs per 8 tiles.
"""

import numpy as np
from contextlib import ExitStack

B = 8
NPOINTS = 8192
EPS = 1e-10
GROUP_FD = 2048
CHUNK = 512
PSUM_BUFS = 2
H16_BUFS = 3    # E strips are [128, n] fp16 = 16KB/partition each


def emit(tc, out_ap, x_ap, y_ap, n=NPOINTS, group_fd=GROUP_FD,
         psum_bufs=PSUM_BUFS, h16_bufs=H16_BUFS, chunk=CHUNK, reps=1,
         drain="both", ablate=None, tag=""):
    """Emit the per-core chamfer kernel into TileContext tc.

    x_ap, y_ap: DRAM [n, 3] f32.  out_ap: DRAM [1, 1] f32.
    """
    import concourse.mybir as mybir
    from concourse.mybir import AluOpType as alu

    nc = tc.nc
    f32 = mybir.dt.float32
    f16 = mybir.dt.float16
    X = mybir.AxisListType.X
    ntile = n // 128
    groups = n // group_fd
    nq = group_fd // chunk

    ctx = ExitStack()
    with ctx:
        singles = ctx.enter_context(tc.tile_pool(name="singles" + tag, bufs=1))
        work = ctx.enter_context(tc.tile_pool(name="work" + tag, bufs=2))
        h16p = ctx.enter_context(tc.tile_pool(name="h16p" + tag, bufs=h16_bufs))
        colp = ctx.enter_context(tc.tile_pool(name="colp" + tag, bufs=2))
        psum = ctx.enter_context(tc.tile_pool(name="psum" + tag, bufs=psum_bufs,
                                              space="PSUM"))

        XW = singles.tile([128, n], f16, tag="XW")
        YW = singles.tile([128, n], f16, tag="YW")
        D1 = singles.tile([128, ntile], f32, tag="D1")
        D2 = singles.tile([128, ntile], f32, tag="D2")
        # Direction-2 accumulator, double-buffered: an in-place
        # tensor_tensor (out aliasing in0) measurably drops the DVE to 1x
        # mode on HW, so the min-accumulate ping-pongs between two buffers.
        ACCa = singles.tile([128, n], f16, tag="ACCa")
        ACCb = singles.tile([128, n], f16, tag="ACCb")
        ping = (ACCa, ACCb)
        junk16 = singles.tile([128, n], f16, tag="junk16")
        pmaxs = singles.tile([128, ntile], f16, tag="pmaxs")
        eps_col = singles.tile([128, 1], f32, tag="eps_col")
        nc.vector.memset(eps_col, EPS)

        # Selection stationaries for the prep gather matmuls.  The per-point
        # vectors live (after transpose) at partitions (t%2)*64 + v, v being
        # the vector-slot index:  [h0 h1 h2 l0 l1 l2 nh nl one, 0...].  The
        # K=16 W-row pattern at psum row i is vec_{vmap[i % 32]} (>=16 -> 0):
        #   XW rows: [xh0..xh2 xl0..xl2 | xh0..xh2 xl0..xl2 | 1 1 | nxh nxl]
        #   YW rows: [yh0..yh2 yl0..yl2 | yl0..yl2 yh0..yh2 | nyh nyl | 1 1]
        # pairing k: 0-2 (xh,yh)d, 3-5 (xl,yl)d, 6-8 (xh,yl)d, 9-11 (xl,yh)d,
        # 12 (1,nyh), 13 (1,nyl), 14 (nxh,1), 15 (nxl,1).
        VMAP_X = [0, 1, 2, 3, 4, 5, 0, 1, 2, 3, 4, 5, 8, 8, 6, 7]
        VMAP_Y = [0, 1, 2, 3, 4, 5, 3, 4, 5, 0, 1, 2, 6, 7, 8, 8]

        # SELX/SELY share one [128, 256] tile filled from one [1, 32*256]
        # content row: SELXY row v = [ SELX cols (4 q-copies of 32) | SELY ].
        # Engines can't write partition-strided single rows, so content is
        # laid out in partition 0 and partition-expanded by one DMA to rows
        # 0..31, then one SBUF->SBUF DMA replicates to base 64 (the only
        # other quadrant base the K=16 stationaries read).
        SELXY = singles.tile([128, 256], f16, tag="SELXY")
        content = singles.tile([1, 32 * 256], f16, tag="selc")
        nc.vector.memset(content, 0.0)
        cw = content[:, :].rearrange("p (v w r) -> p v w r", w=8, r=32)
        for off, vmap in ((0, VMAP_X), (4, VMAP_Y)):
            by_v = {}
            for r, v in enumerate(vmap):
                by_v.setdefault(v, []).append(r)
            for v, rs in by_v.items():
                start = prev = rs[0]
                for r in rs[1:] + [None]:
                    if r is not None and r == prev + 1:
                        prev = r
                        continue
                    nc.vector.memset(cw[:, v, off:off + 4, start:prev + 1], 1.0)
                    if r is not None:
                        start = prev = r
        nc.sync.dma_start(out=SELXY[0:32, :], in_=content[0:1, :])
        nc.sync.dma_start(out=SELXY[64:96, :], in_=SELXY[0:32, :])
        SELX = SELXY[:, 0:128]
        SELY = SELXY[:, 128:256]

        def prep(inp, W, sel):
            """Build W [128, n] fp16 (16 K-rows replicated at partition
            bases 0/64) without partition-collapsing DMAs: compute the
            9 per-point vectors, transpose them with one xbar DMA, then
            gather+replicate into W via selection matmuls + PSUM cast."""
            # contiguous load: partition p holds points p*ntile + t,
            # cols (t d).  Point identity is a pure relabeling that the
            # min/mean reductions never observe.
            Xw = work.tile([128, 3 * ntile], f32, tag="Xw")
            nc.sync.dma_start(
                out=Xw,
                in_=inp.rearrange("(p t) d -> p (t d)", p=128),
            )
            Xw3 = Xw[:, :].rearrange("p (t d) -> p d t", d=3)
            # V: 64 vector slots per tile, col order t*64 + v.  64 slots (not
            # 32) so the transposed slot rows land at partition bases {0, 64}
            # only: 32/96-base PE tiles fail at runtime on this silicon.
            V = work.tile([128, 64 * ntile], f16, tag="V")
            nc.vector.memset(V, 0.0)
            v32 = V[:, :].rearrange("p (t v) -> p v t", v=64)
            nc.vector.tensor_copy(v32[:, 0:3, :], Xw3)               # xh
            nc.vector.tensor_tensor(v32[:, 3:6, :], Xw3, v32[:, 0:3, :],
                                    alu.subtract)                    # xl
            Sq = work.tile([128, 3 * ntile], f32, tag="Sq")
            nc.vector.tensor_tensor(Sq, Xw, Xw, alu.mult)
            sq3 = Sq[:, :].rearrange("p (t d) -> p d t", d=3)
            nxx = work.tile([128, ntile], f32, tag="nxx")
            nc.vector.tensor_tensor(nxx, sq3[:, 0, :], sq3[:, 1, :], alu.add)
            nc.vector.tensor_tensor(nxx, nxx, sq3[:, 2, :], alu.add)
            nc.vector.tensor_scalar_mul(nxx, nxx, -0.5)
            nc.scalar.copy(v32[:, 6, :], nxx)                        # nh
            nc.vector.tensor_tensor(v32[:, 7, :], nxx, v32[:, 6, :],
                                    alu.subtract)                    # nl
            nc.vector.memset(v32[:, 8, :], 1.0)                      # ones
            if ablate == "prepV":
                return V
            # transpose: TV[(t%2)*64 + v, (t//2)*128 + p] = vec_v[point(p,t)]
            # (one xbar DMA; it transposes 128-col tiles independently).
            TV = work.tile([128, (ntile // 2) * 128], f16, tag="TV")
            nc.sync.dma_start_transpose(
                TV[:, :].rearrange("p (c j) -> p c j", j=128), V)
            if ablate == "prepTV":
                return TV
            # gather+replicate via selection matmuls, cast PSUM -> W.  Each
            # matmul streams a [16, 512] moving block into one full PSUM
            # bank: TV's base-0 rows hold the even tiles of an 8-tile group
            # contiguously, base-64 rows the odd tiles.  W's 128-col blocks
            # come out in (even..., odd...) permuted tile order — again a
            # pure point relabeling.
            for c in range(n // group_fd):
                ps = psum.tile([128, group_fd], f32, tag="ps")
                for j in range(group_fd // 512):
                    b = c * (group_fd // 512) + j
                    q = 64 * (b % 2)
                    nc.tensor.matmul(
                        ps[:, j * 512:(j + 1) * 512],
                        sel[q:q + 16, :],
                        TV[q:q + 16, (b // 2) * 512:(b // 2) * 512 + 512],
                        start=True, stop=True, tile_position=(q, 0),
                    )
                if c % 2 == 0:
                    nc.scalar.copy(W[:, c * group_fd:(c + 1) * group_fd], ps)
                else:
                    nc.vector.tensor_copy(W[:, c * group_fd:(c + 1) * group_fd],
                                          ps)

        def dbg_out(src_f16):
            dbg = singles.tile([1, 1], f32, tag="dbg")
            nc.vector.tensor_copy(dbg, src_f16)
            nc.sync.dma_start(out=out_ap, in_=dbg)

        if ablate == "sel":
            dbg_out(SELXY[0:1, 0:1])
            return
        if ablate in ("prepV", "prepTV"):
            probe = prep(x_ap, XW, SELX)
            dbg_out(probe[0:1, 0:1])
            return
        prep(x_ap, XW, SELX)
        if ablate == "prep1":
            dbg_out(XW[0:1, 0:1])
            return
        prep(y_ap, YW, SELY)
        if ablate == "prep2":
            dbg_out(YW[0:1, 0:1])
            return

        # ---- main sweep: one matmul pass ----
        # Per t-tile: PE fills 4 PSUM groups (2-quadrant K=16 matmuls,
        # 512-col bank-aligned chunks); ACT alone drains each PSUM group
        # with a Relu(-x) activation — E' = relu(-P) = d^2/2 clamped >= 0,
        # so the sqrt guard costs nothing — casting fp16 quarters into one
        # contiguous [128, n] strip (the ACCa tile at t=0, an E tile
        # otherwise).  DVE then runs ONE whole-strip row-MIN
        # (tensor_scalar junk write at 4x rate; accum_out into a small colp
        # tile: direction 1 needs only the min over ALL m, so per-group
        # granularity would just add sync edges) and ONE whole-strip
        # min-accumulate into the ping-pong accumulator (t>0).  Row minima
        # batch 8-at-a-time before one copy out to pmaxs (runtime rejects
        # accum_out columns at large offsets).
        relu = mybir.ActivationFunctionType.Relu
        gi = 0
        for _rep in range(reps):
          for t in range(ntile):
              strip = ping[0] if t == 0 else h16p.tile([128, n], f16, tag="E")
              for g in range(groups):
                  ps = psum.tile([128, group_fd], f32, tag="ps")
                  for c in range(nq):
                      m0 = g * group_fd + c * chunk
                      qi = gi * nq + c
                      q = 64 * (qi % 2)
                      nc.tensor.matmul(
                          ps[:, c * chunk:(c + 1) * chunk],
                          XW[q:q + 16, t * 128:(t + 1) * 128],
                          YW[q:q + 16, m0:m0 + chunk],
                          start=True, stop=True,
                          tile_position=(q, 0),
                      )
                  gi += 1
                  nc.scalar.activation(
                      strip[:, g * group_fd:(g + 1) * group_fd], ps[:, :],
                      relu, scale=-1.0)
              if drain == "actonly":
                  continue
              if t % 8 == 0:
                  pm8 = colp.tile([128, 8], f16, tag="pm8")
              nc.vector.tensor_scalar(junk16, strip, 0.0, None,
                                      alu.max, alu.min,
                                      accum_out=pm8[:, t % 8:t % 8 + 1])
              if t % 8 == 7:
                  nc.vector.tensor_copy(pmaxs[:, t - 7:t + 1], pm8)
              if t > 0:
                  nc.vector.tensor_tensor(ping[t % 2], ping[(t + 1) % 2],
                                          strip, alu.min)

        if drain == "actonly":      # timing probe: no direction-1 state
            dbg_out(ping[0][0:1, 0:1])
            return
        ACC = ping[(ntile - 1) % 2]

        # D1 = sqrt(2*min + EPS), one activation for all tiles (minima are
        # >= 0 by the Relu cast).
        nc.scalar.activation(D1[:, :], pmaxs[:, :],
                             mybir.ActivationFunctionType.Sqrt,
                             bias=eps_col[:, :], scale=2.0)

        # ---- direction-2 tail: transpose ACC once, reduce, sqrt ----
        GB = colp.tile([128, ntile], f16, tag="GB")
        tch = 2048
        for g in range(n // tch):
            tp = h16p.tile([128, tch], f16, tag="tp")
            nc.sync.dma_start_transpose(
                tp[:, :].rearrange("p (c j) -> p c j", j=128),
                ACC[:, g * tch:(g + 1) * tch])
            nc.vector.tensor_reduce(
                GB[:, g * (tch // 128):(g + 1) * (tch // 128)],
                tp[:, :].rearrange("p (c j) -> p c j", j=128),
                axis=X, op=alu.min)
        nc.scalar.activation(D2[:, :], GB[:, :],
                             mybir.ActivationFunctionType.Sqrt,
                             bias=eps_col[:, :], scale=2.0)

        # ---- mean over points, max of the two directions, write out ----
        sums = singles.tile([128, 2], f32, tag="sums")
        nc.vector.tensor_reduce(sums[:, 0:1], D1[:, :], axis=X, op=alu.add)
        nc.vector.tensor_reduce(sums[:, 1:2], D2[:, :], axis=X, op=alu.add)
        ones = singles.tile([128, 1], f32, tag="ones")
        nc.vector.memset(ones, 1.0)
        pstail = psum.tile([128, group_fd], f32, tag="ps")
        pq = pstail[0:1, 0:2]
        nc.tensor.matmul(pq, ones[:, :], sums[:, :], start=True, stop=True)
        fin = singles.tile([1, 2], f32, tag="fin")
        res = singles.tile([1, 1], f32, tag="res")
        nc.vector.tensor_scalar(fin, pq, 1.0 / n, None, alu.mult, alu.max,
                                accum_out=res)
        nc.sync.dma_start(out=out_ap, in_=res)


_NC_CACHE = {}


def build(n=NPOINTS, reps=1, body_reps=1, group_fd=GROUP_FD,
          psum_bufs=PSUM_BUFS, h16_bufs=H16_BUFS, chunk=CHUNK, drain="both",
          ablate=None):
    key = (n, reps, body_reps, group_fd, psum_bufs, h16_bufs, chunk, drain,
           ablate)
    if key in _NC_CACHE:
        return _NC_CACHE[key]
    import concourse.mybir as mybir
    import concourse.tile as tile
    from concourse import bacc

    nc = bacc.Bacc(None, target_bir_lowering=False)
    x = nc.dram_tensor("x", [n, 3], mybir.dt.float32, kind="ExternalInput")
    y = nc.dram_tensor("y", [n, 3], mybir.dt.float32, kind="ExternalInput")
    out = nc.dram_tensor("out", [1, 1], mybir.dt.float32, kind="ExternalOutput")
    with tile.TileContext(nc) as tc:
        for r in range(body_reps):
            emit(tc, out[:, :], x[:, :], y[:, :], n=n, group_fd=group_fd,
                 psum_bufs=psum_bufs, h16_bufs=h16_bufs, chunk=chunk,
                 reps=reps, drain=drain, ablate=ablate,
                 tag=f"r{r}" if r else "")
    nc.finalize()
    _NC_CACHE[key] = nc
    return nc


def kernel(x, y):
    """Full-input entry point: x, y (8, 8192, 3) f32 -> scalar f32."""
    from concourse.bass_utils import run_bass_kernel_spmd

    x = np.asarray(x, dtype=np.float32)
    y = np.asarray(y, dtype=np.float32)
    assert x.shape == (B, NPOINTS, 3) and y.shape == (B, NPOINTS, 3)
    nc = build()
    in_maps = [
        {"x": np.ascontiguousarray(x[b]), "y": np.ascontiguousarray(y[b])}
        for b in range(B)
    ]
    res = run_bass_kernel_spmd(nc, in_maps, core_ids=list(range(B)))
    total = np.float32(0.0)
    for r in res.results:
        total = np.float32(total + np.float32(r["out"][0, 0]))
    return total
